# revision 59
# baseline (speedup 1.0000x reference)
"""Distributed Trainium2 kernel for nn_DecoderAttentionRotary.

Strategy (8 NeuronCores, tensor-parallel over heads, fp16 matmul datapath):
  - host: transpose x -> xT [D, B*L] fp16; per-core Wqkv column slice
    reordered to [q0,k0,q1,k1,v0|v1] fp16; cos/sin transposed+batch-tiled;
    causal masks fp16.
  - device, per core (2 heads):
      phase 1 (both batches): per-kt weight tiles (fast start), qkT =
        (Wqk^T @ xT) + b, v = x @ Wv in [l, hd] layout, RoPE fused per
        512-col chunk. Wd + bias prefetched on the gpsimd DMA queue.
      phase 2: causal attention in scores^T layout, software-pipelined:
        score mms for group g+1 issue before outp/sump mms of group g so
        the PE never waits on the exp; exp batched as [128,1024] ACT ops;
        normalize chain kept off the PE (psum->sbuf copies release banks
        early; reciprocal + gpsimd broadcast pipelined).
        Per-(batch,head) AllToAll reshard (fp16) so collectives fire at
        head granularity and hide under later compute.
      phase 3: y rows = outT_rows^T @ Wd + bd; per-kt o_sb pieces loaded
        as each A2A lands; Wd fully SBUF-resident (loaded during ph1).
  - host: scatter the per-core 256-row halves into the full output.
"""
import sys

for _p in ("/opt/pypackages", "/opt/trn_rl_repo"):
    if _p not in sys.path:
        sys.path.insert(0, _p)

import numpy as np

B, L, D, H = 2, 2048, 2048, 16
HD, R = 128, 32
SCALE = float(HD) ** -0.5
W = 8
HPC = H // W              # heads per core
M = B * L                 # flattened rows
CORES = list(range(W))

_NC = None


def _build_nc():
    import concourse.mybir as mybir
    import concourse.tile as tile
    from concourse import bacc

    f32 = mybir.dt.float32
    f16 = mybir.dt.float16
    AFT = mybir.ActivationFunctionType
    OP = mybir.AluOpType

    nc = bacc.Bacc(None, target_bir_lowering=False, num_devices=W)
    xT = nc.declare_dram_parameter("xT", [D, M], f16, isOutput=False)
    wqkv = nc.declare_dram_parameter("wqkv", [D, 6 * HD], f16, isOutput=False)
    bqk = nc.declare_dram_parameter("bqk", [4 * HD, 1], f32, isOutput=False)
    bv = nc.declare_dram_parameter("bv", [1, 2 * HD], f16, isOutput=False)
    cs2 = nc.declare_dram_parameter("cs2", [R, 2, M], f16, isOutput=False)
    ident = nc.declare_dram_parameter("ident", [128, 128], f16, isOutput=False)
    mneg = nc.declare_dram_parameter("mneg", [128, 128], f16, isOutput=False)
    wd = nc.declare_dram_parameter("wd", [D, D], f16, isOutput=False)
    bdb = nc.declare_dram_parameter("bdb", [128, D], f16, isOutput=False)
    onesc = nc.declare_dram_parameter("onesc", [128, 1], f16, isOutput=False)
    y = nc.declare_dram_parameter("y", [M // W, D], f32, isOutput=True)

    xT_r = xT.ap().rearrange("(t p) n -> p t n", p=128)   # [128, 16, M]
    wq_r = wqkv.ap().rearrange("(t p) m -> p t m", p=128)
    wd_r = wd.ap().rearrange("(t p) n -> p t n", p=128)

    NCH = L // 512            # 4 chunks per batch
    NQ = 4                    # xt quarters per chunk

    with tile.TileContext(nc) as tc:
        with (
            nc.allow_low_precision(reason="fp16 pipeline checked vs reference"),
            tc.tile_pool(name="const", bufs=1) as cpool,
            tc.tile_pool(name="dram", bufs=1, space="DRAM") as dpool,
            tc.tile_pool(name="ps", bufs=1, space="PSUM") as pp,
            tc.tile_pool(name="qkv", bufs=1) as qkvpool,
            tc.tile_pool(name="wdp", bufs=1) as wdpool,
            tc.tile_pool(name="att", bufs=3) as apool,
            tc.tile_pool(name="p1", bufs=3) as p1pool,
        ):
            a2a_ins = [[dpool.tile([W, HD, 256], f16, name=f"a2ain{b}_{h}")
                        for h in range(HPC)] for b in range(B)]
            a2a_outs = [[dpool.tile([W, HD, 256], f16, name=f"a2aout{b}_{h}")
                         for h in range(HPC)] for b in range(B)]

            # ---- startup DMAs: first xt quarters interleaved with w tiles
            # so the first matmul fires after ~2us, not after all weights.
            def xt_quarter(b, nch, q):
                t = p1pool.tile([128, 4, 512], f16, tag="xt", bufs=6)
                n0 = b * L + nch * 512
                nc.sync.dma_start(
                    out=t[:], in_=xT_r[:, 4 * q:4 * (q + 1), n0:n0 + 512]
                )
                return t

            w_t = []
            xt_cur = []
            for q in range(NQ):
                xt_cur.append(xt_quarter(0, 0, q))
                for kt in range(4 * q, 4 * q + 4):
                    wt = cpool.tile([128, 6 * HD], f16, name=f"w{kt}")
                    nc.sync.dma_start(out=wt[:], in_=wq_r[:, kt, :])
                    w_t.append(wt)
            bqk_sb = cpool.tile([128, 4], f32)
            nc.sync.dma_start(
                out=bqk_sb[:], in_=bqk.ap().rearrange("(t p) o -> p (t o)", p=128)
            )
            bv_sb = cpool.tile([1, 2 * HD], f16)
            nc.sync.dma_start(out=bv_sb[:], in_=bv.ap())
            ones_r = cpool.tile([1, 128], f16)
            nc.vector.memset(ones_r[:], 1.0)
            ones_c = cpool.tile([128, 1], f16)
            nc.sync.dma_start(out=ones_c[:], in_=onesc.ap())
            id_sb = cpool.tile([128, 128], f16)
            nc.sync.dma_start(out=id_sb[:], in_=ident.ap())
            mn_sb = cpool.tile([128, 128], f16)
            nc.sync.dma_start(out=mn_sb[:], in_=mneg.ap())
            neg_row = cpool.tile([1, 384], f16)
            nc.vector.memset(neg_row[:], -30000.0)

            # output bias prefetch (small); Wd tiles are declared here but
            # their loads are paced inside attention b0 (scalar queue) so
            # the 16.4MB doesn't fight phase 1's xt stream for HBM
            bd_sb = cpool.tile([128, D], f16)
            nc.gpsimd.dma_start(out=bd_sb[:], in_=bdb.ap())
            wd_t = [wdpool.tile([128, D], f16, name=f"wd{kt}")
                    for kt in range(16)]

            qk_sbs, v_sbs = [], []
            for b in range(B):
                qk_sbs.append(qkvpool.tile([128, 4, L], f16, name=f"qk{b}"))
                v_sbs.append(qkvpool.tile([128, 16, 2 * HD], f16, name=f"v{b}"))

            # ---- phase 1 (both batches) + fused RoPE ----
            for b in range(B):
                qk_sb, v_sb = qk_sbs[b], v_sbs[b]
                for nch in range(NCH):
                    n0 = b * L + nch * 512
                    ch = slice(nch * 512, (nch + 1) * 512)
                    if b == 0 and nch == 0:
                        xts = xt_cur
                    # prefetch next chunk's first-half quarters (the bufs=6
                    # rotation targets are free); q2/q3 issue after this
                    # chunk's V matmuls so their rotation has no forward
                    # hazard on this chunk's reads
                    nb, nn = (b, nch + 1) if nch + 1 < NCH else (b + 1, 0)
                    xt_next = None
                    if nb < B:
                        xt_next = [xt_quarter(nb, nn, q) for q in range(2)]
                    # per-chunk cos/sin slices ([:,0,:] cos, [:,1,:] sin)
                    # on the DVE queue: keeps the sync queue free for xt
                    cs_c = p1pool.tile([R, 2, 512], f16, tag="csc", bufs=2)
                    nc.scalar.dma_start(out=cs_c[:], in_=cs2.ap()[:, :, n0:n0 + 512])

                    for mp in range(2):
                        ps = pp.tile([128, 1024], f32, tag="sc", bufs=3,
                                     name=f"qkps{b}_{nch}_{mp}")
                        for kt in range(16):
                            xt = xts[kt // 4]
                            for i in range(2):
                                m = 2 * mp + i
                                nc.tensor.matmul(
                                    ps[:, i * 512:(i + 1) * 512],
                                    lhsT=w_t[kt][:, m * 128:(m + 1) * 128],
                                    rhs=xt[:, kt % 4, :],
                                    start=(kt == 0),
                                    stop=(kt == 15),
                                )
                        for i in range(2):
                            m = 2 * mp + i
                            nc.vector.tensor_scalar_add(
                                qk_sb[:, m, ch], ps[:, i * 512:(i + 1) * 512],
                                bqk_sb[:, m:m + 1],
                            )
                    for m in range(4):
                        # fused RoPE on rows 0:R of this chunk
                        ta = p1pool.tile([R, 512], f16, tag="ta", bufs=2)
                        rot = p1pool.tile([R, 512], f16, tag="rot", bufs=2)
                        nc.scalar.dma_start(out=rot[0:16, :], in_=qk_sb[16:32, m, ch])
                        nc.scalar.dma_start(out=rot[16:32, :], in_=qk_sb[0:16, m, ch])
                        nc.vector.tensor_tensor(
                            ta[:], qk_sb[0:R, m, ch], cs_c[:, 0, :], op=OP.mult
                        )
                        tb = p1pool.tile([R, 512], f16, tag="tb", bufs=2)
                        nc.vector.tensor_tensor(
                            tb[:], rot[:], cs_c[:, 1, :], op=OP.mult
                        )
                        nc.vector.tensor_tensor(
                            qk_sb[0:R, m, ch], ta[:], tb[:], op=OP.add
                        )
                    for rr2 in range(2):
                        # vpss pair shares one sc tile: slices sit in the
                        # tile's two separate PSUM banks (independent
                        # accumulation groups must not share a bank)
                        vps = pp.tile([128, 1024], f32, tag="sc", bufs=3,
                                      name=f"vps{b}_{nch}_{rr2}")
                        vsl = [slice(0, 2 * HD), slice(512, 512 + 2 * HD)]
                        for kt in range(16):
                            xt = xts[kt // 4]
                            for i in range(2):
                                rr = 2 * rr2 + i
                                nc.tensor.matmul(
                                    vps[:, vsl[i]],
                                    lhsT=xt[:, kt % 4, rr * 128:(rr + 1) * 128],
                                    rhs=w_t[kt][:, 4 * HD:6 * HD],
                                    start=(kt == 0),
                                    stop=False,
                                )
                        for i in range(2):
                            rr = 2 * rr2 + i
                            nc.tensor.matmul(
                                vps[:, vsl[i]], lhsT=ones_r[:], rhs=bv_sb[:],
                                start=False, stop=True,
                            )
                            nc.scalar.activation(
                                v_sb[:, nch * 4 + rr, :], vps[:, vsl[i]],
                                AFT.Copy,
                            )
                    if xt_next is not None:
                        xt_next += [xt_quarter(nb, nn, q) for q in range(2, NQ)]
                    xts = xt_next

            # ---- phase 2: attention; A2A per (batch, head) ----
            osb_tiles = {}

            def osb_load(bh, kt):
                # o_sb piece (j, u): dims [j*256+u*128 : +128) of attnout^T
                j, u = kt // 2, kt % 2
                t = apool.tile([128, 256], f16, tag="osb", bufs=16)
                nc.gpsimd.dma_start(out=t[:], in_=a2a_outs[bh][u][j, :, :])
                osb_tiles[(bh, kt)] = t

            for b in range(B):
                qk_sb, v_sb = qk_sbs[b], v_sbs[b]
                for h in range(HPC):
                    # software pipeline GLOBAL across qc blocks: outp/sump
                    # matmuls trail the score matmuls by two groups even
                    # across block boundaries, so the PE never drains and
                    # the clock gate stays at full rate
                    pend = []

                    def drain(n):
                        while len(pend) > n:
                            emit, et, kis = pend.pop(0)
                            emit(et, kis)

                    def norm_block(qc, outp, sump):
                        # normalize: fast psum->sbuf copies release the acc
                        # banks; recip+bcast+mult pipeline off the PE path
                        osum = apool.tile([128, 512], f16, tag="osum", bufs=2)
                        nc.vector.tensor_copy(osum[:], outp[:])
                        ssum = apool.tile([1, 512], f16, tag="ssum", bufs=2)
                        nc.vector.tensor_copy(ssum[:], sump[0:1, :])
                        rec = apool.tile([1, 512], f16, tag="rec", bufs=2)
                        nc.vector.reciprocal(rec[:], ssum[:])
                        bcs = apool.tile([128, 512], f16, tag="bcs", bufs=2)
                        nc.gpsimd.partition_broadcast(bcs[:], rec[:])
                        ot = apool.tile([128, 512], f16, tag="ot", bufs=2)
                        nc.gpsimd.tensor_tensor(ot[:], osum[:], bcs[:],
                                                op=OP.mult)
                        for half in range(2):
                            nc.sync.dma_start(
                                out=a2a_ins[b][h][2 * qc + half, :, :],
                                in_=ot[:, half * 256:(half + 1) * 256],
                            )
                        # pace Wd residency loads on the sync queue (idle
                        # during attention apart from small ot writes):
                        # 1 tile per b0 block, 2 per b1-h0 block
                        if b == 0:
                            wk = 4 * h + (3 - qc)
                            nc.sync.dma_start(out=wd_t[wk][:],
                                              in_=wd_r[:, wk, :])
                        elif h == 0:
                            wk = 8 + 2 * (3 - qc)
                            for kt in (wk, wk + 1):
                                nc.sync.dma_start(out=wd_t[kt][:],
                                                  in_=wd_r[:, kt, :])

                    post = []   # (qc, outp, sump) awaiting normalize
                    for qc in reversed(range(NCH)):
                        nk = 4 * qc + 4
                        outp = pp.tile([128, 512], f32, tag="acc", bufs=2,
                                       name=f"outp{b}_{h}_{qc}")
                        sump = pp.tile([128, 512], f32, tag="acc", bufs=2,
                                       name=f"sump{b}_{h}_{qc}")

                        def emit_av(et, kis, outp=outp, sump=sump, nk=nk,
                                    qc=qc):
                            # et tail columns are exact zeros (mask fill),
                            # so full-width matmuls and a pair-summed single
                            # sump matmul per group are safe
                            ets = apool.tile([128, 512], f16, tag="ets",
                                             bufs=2)
                            nc.vector.tensor_tensor(
                                ets[:], et[:, 0:512], et[:, 512:1024],
                                op=OP.add,
                            )
                            for idx, ki in enumerate(kis):
                                nc.tensor.matmul(
                                    outp[:],
                                    lhsT=v_sb[:, ki, h * 128:(h + 1) * 128],
                                    rhs=et[:, idx * 512:idx * 512 + 512],
                                    start=(ki == 0), stop=(ki == nk - 1),
                                )
                            nc.tensor.matmul(
                                sump[0:1, :], lhsT=ones_c[:], rhs=ets[:],
                                start=(kis[0] == 0), stop=(kis[1] == nk - 1),
                            )
                            if kis[1] == nk - 1:
                                post.append((qc, outp, sump))

                        for g0 in range(0, nk, 2):
                            kis = [g0, g0 + 1]
                            sc2 = pp.tile([128, 1024], f32, tag="sc", bufs=3,
                                          name=f"sp{b}_{h}_{qc}_{g0}")
                            for idx, ki in enumerate(kis):
                                c0 = max(0, ki - qc * 4) * 128
                                npr = 512 - c0
                                diag = ki >= qc * 4
                                s = idx * 512
                                # q-aligned: et col j always holds q col
                                # qc*512+j, so full-width outp matmuls and
                                # pair-summed sump stay consistent
                                nc.tensor.matmul(
                                    sc2[:, s + c0:s + 512],
                                    lhsT=qk_sb[:, 2 * h + 1,
                                               ki * 128:(ki + 1) * 128],
                                    rhs=qk_sb[:, 2 * h,
                                              qc * 512 + c0:(qc + 1) * 512],
                                    start=True,
                                    stop=not (diag or c0 > 0),
                                )
                                if diag:
                                    # causal mask: add -30000 to the upper
                                    # triangle of the boundary 128 cols so
                                    # the exp underflows to zero (no vector
                                    # engine pass over et needed)
                                    nc.tensor.matmul(
                                        sc2[:, s + c0:s + c0 + 128],
                                        lhsT=id_sb[:],
                                        rhs=mn_sb[:], start=False,
                                        stop=(c0 == 0),
                                    )
                                if c0 > 0:
                                    # fill the skipped (fully-masked) column
                                    # range with -30000 so exp zeroes it
                                    nc.tensor.matmul(
                                        sc2[:, s:s + c0],
                                        lhsT=ones_r[:],
                                        rhs=neg_row[0:1, 0:c0],
                                        start=False, stop=True,
                                    )
                            et = apool.tile([128, 1024], f16, tag="et", bufs=3)
                            nc.scalar.activation(et[:], sc2[:], AFT.Exp,
                                                 scale=SCALE)
                            pend.append((emit_av, et, kis))
                            drain(2)
                            while post:
                                norm_block(*post.pop(0))
                    drain(0)
                    while post:
                        norm_block(*post.pop(0))
                    nc.gpsimd.collective_compute(
                        "AllToAll",
                        mybir.AluOpType.bypass,
                        replica_groups=[CORES],
                        ins=[a2a_ins[b][h][:]],
                        outs=[a2a_outs[b][h][:]],
                    )
                    # o_sb piece loads, placed so the gpsimd queue never
                    # blocks on an unfinished collective ahead of live work:
                    if b == 1 and h == 0:
                        for kt in range(16):
                            osb_load(0, kt)          # b0 pieces (a2a done)

            # ---- phase 3: output projection, b=0 rows then b=1 rows ----
            kt_order = [2 * j for j in range(8)] + [2 * j + 1 for j in range(8)]
            for bh in range(2):
                if bh == 1:
                    # b1 pieces load during ph3-bh0 (bufs=16 rotation is
                    # legal here: all bh0 readers are already emitted)
                    for kt in kt_order:
                        osb_load(1, kt)
                for n4 in range(4):
                    yp = pp.tile([128, 1024], f32, tag="sc", bufs=3,
                                 name=f"yps{bh}_{n4}")
                    for kk, kt in enumerate(kt_order):
                        for i in range(2):
                            nc.tensor.matmul(
                                yp[:, i * 512:(i + 1) * 512],
                                lhsT=osb_tiles[(bh, kt)][:,
                                                         i * 128:(i + 1) * 128],
                                rhs=wd_t[kt][:, n4 * 512:(n4 + 1) * 512],
                                start=(kk == 0), stop=(kk == 15),
                            )
                    for i in range(2):
                        m = 2 * bh + i
                        yt = apool.tile([128, 512], f32, tag="yt", bufs=2)
                        nc.vector.tensor_tensor(
                            yt[:], yp[:, i * 512:(i + 1) * 512],
                            bd_sb[:, n4 * 512:(n4 + 1) * 512], op=OP.add,
                        )
                        nc.sync.dma_start(
                            out=y[m * 128:(m + 1) * 128,
                                  n4 * 512:(n4 + 1) * 512],
                            in_=yt[:],
                        )
    nc.finalize()
    return nc


def _host_prep(x_BLD, cos, sin, Wqkv, bqkv, Wd, bd):
    x = np.asarray(x_BLD, np.float32).reshape(M, D)
    xT = np.ascontiguousarray(x.T.astype(np.float16))
    c2 = np.asarray(cos, np.float32).reshape(L, R).T
    s2 = np.asarray(sin, np.float32).reshape(L, R).T
    cosT = np.ascontiguousarray(np.concatenate([c2] * B, axis=1))
    sinT = np.concatenate([s2] * B, axis=1)
    sinT_pm = np.ascontiguousarray(
        np.concatenate([-sinT[:16], sinT[16:]], axis=0)
    )
    kk = np.arange(128, dtype=np.int64)[:, None]
    qq = np.arange(128, dtype=np.int64)[None, :]
    mneg = np.where(qq < kk, np.float16(-30000.0), np.float16(0.0))
    bdb = np.ascontiguousarray(
        np.broadcast_to(np.asarray(bd, np.float16), (128, D))
    )
    Wqkv = np.asarray(Wqkv, np.float32)
    bqkv = np.asarray(bqkv, np.float32)
    in_maps = []
    for c in range(W):
        base = c * HPC * 3 * HD
        qk_idx = np.concatenate(
            [np.arange(base + h * 3 * HD, base + h * 3 * HD + 2 * HD)
             for h in range(HPC)]
        )
        v_idx = np.concatenate(
            [np.arange(base + h * 3 * HD + 2 * HD, base + (h + 1) * 3 * HD)
             for h in range(HPC)]
        )
        in_maps.append({
            "xT": xT,
            "wqkv": np.ascontiguousarray(
                Wqkv[:, np.concatenate([qk_idx, v_idx])].astype(np.float16)
            ),
            "bqk": np.ascontiguousarray(bqkv[qk_idx].reshape(4 * HD, 1)),
            "bv": np.ascontiguousarray(
                bqkv[v_idx].reshape(1, 2 * HD).astype(np.float16)
            ),
            "cs2": np.ascontiguousarray(
                np.stack([cosT, sinT_pm], axis=1).astype(np.float16)
            ),
            "ident": np.eye(128, dtype=np.float16),
            "mneg": mneg.astype(np.float16),
            "wd": np.asarray(Wd, np.float32).astype(np.float16),
            "bdb": bdb,
            "onesc": np.ones((128, 1), np.float16),
        })
    return in_maps


def _get_nc():
    global _NC
    if _NC is None:
        _NC = _build_nc()
    return _NC


def _run(inputs, trace=False, tmpdir=None):
    from concourse.bass_utils import run_bass_kernel_spmd

    in_maps = _host_prep(**inputs)
    nc = _get_nc()
    res = run_bass_kernel_spmd(nc, in_maps, CORES, trace=trace, tmpdir=tmpdir)
    out = np.empty((M, D), np.float32)
    for c in CORES:
        yc = res.results[c]["y"]          # [512, D]: rows b0 then b1
        out[c * 256:(c + 1) * 256] = yc[:256]
        out[L + c * 256:L + (c + 1) * 256] = yc[256:]
    return out.reshape(B, L, D), res


def kernel(**inputs) -> np.ndarray:
    out, _ = _run(inputs)
    return out
